# revision 21
# baseline (speedup 1.0000x reference)
"""ClassAwareKnittingMemoryV2 Trainium2 kernel (8 NeuronCores, data-parallel over B).

Math restructure vs the reference:
  - query/attention computed fully "transposed" (feature-major [C, L]); host
    transposes x (and transposes back the outputs), so the device needs no
    transposes at all.
  - cat_mem = shared_memory + delta, so per-category keys/values split as
      kcT = skT + Wk^T @ deltaT,   vc = sv + deltaT^T @ Wv
    with skT/sv computed once (B-independent) and deltaT computed c-major
    directly via 256 tiny bf16 matmuls:
      deltaT[c_win, (b,v)] = (cat_B window [80, 128])^T @ a_blockdiag [80, 40]
    (windows are pure-c because cat_B's free dim is (s, c) with c inner).
  - Scores are computed transposed ([keys, L]); softmax runs over the
    partition dim: exp (no max subtraction -- scaled scores are within +/-3),
    per-scale visibility handled by a 0/1 mask multiply, sums via ones-matmul,
    and the (1-alpha)/sum_s resp. alpha/sum_c normalization factors are
    broadcast to 128 partitions with a K=1 ones-matmul and multiplied into the
    exp tiles.  Both attention outputs then accumulate into one PSUM group:
      memT = sv^T @ (exp_s * f1) + vc^T @ (exp_c * f2),  f1=(1-a)/S_s, f2=a/S_c
  - alpha MLP: aW1 splits into x/ce/scale parts; ce/scale parts and all the
    tiny gathers (cat_A[cid], cat_emb[cid]) are host-side; sigmoid gate of an
    invalid (CFG) category is forced ~0 by biasing ab2 to -30.
  - sigmoid(gk/gv) gates and temp are folded into host-prepared weights.

Matmul dtype: float32r (full PE rate at N>=256, ~1.5e-4 rel err) everywhere
except the delta pipeline (bf16; delta is ~30x smaller than shared_memory so
bf16 there contributes ~1e-4 relative).
"""

import math

import numpy as np

import concourse.bass as bass
import concourse.tile as tile
from concourse import bacc, mybir
from concourse.bass_utils import run_bass_kernel_spmd

# ---- problem constants (hardcoded per contract) ----
B, C, L = 32, 1024, 680
S, SLOTS, R, NCAT = 10, 32, 8, 22
VS = S * SLOTS  # 320
MEM_RANK, AH = 64, 128
PATCH_NUMS = (1, 2, 3, 4, 5, 6, 8, 10, 13, 16)
_lens = [p * p for p in PATCH_NUMS]
_cum = np.concatenate([[0], np.cumsum(_lens)])
BEGIN_ENDS = [(int(_cum[i]), int(_cum[i + 1])) for i in range(S)]

NCORES = 8
BL = B // NCORES  # 4 batch elements per core
P = 128
CO = C // P  # 8 c-chunks
NT = [(0, 340), (340, 340)]  # (start, width) l n-tiles (>=256: fp32r full rate)
VS_CHUNKS = [(0, 128), (128, 128), (256, 64)]  # (start, rows) of key chunks

F32 = mybir.dt.float32
R32 = mybir.dt.float32r
BF16 = mybir.dt.bfloat16

import ml_dtypes

_nc_cache = {}
_last_in_maps = None


def _build(sc_scale: float) -> bass.Bass:
    nc = bacc.Bacc("TRN2", target_bir_lowering=False, debug=False,
                   num_devices=NCORES)

    def din(name, shape, dt=R32):
        return nc.dram_tensor(name, shape, dt, kind="ExternalInput")

    xt = din("xt", [BL, C, L])
    smt = din("smt", [C, VS])
    wq = din("wq", [C, C])
    wk = din("wk", [C, C])
    wv = din("wv", [C, C])
    wkb = din("wkb", [C, C], BF16)
    wvb = din("wvb", [C, C], BF16)
    catb = din("catb", [S * R, SLOTS * C], BF16)
    abd = din("abd", [S * R, BL * S], BF16)
    aw1x = din("aw1x", [C, AH])
    aw2 = din("aw2", [AH, 1])
    biasall = din("biasall", [AH, BL, S], F32)
    kw1v1 = din("kw1v1", [C, 128])
    kb1v1 = din("kb1v1", [128, 1], F32)
    kw2v2 = din("kw2v2", [128, C])
    kb2s = din("kb2s", [P, CO], F32)
    vb2s = din("vb2s", [P, CO], F32)
    ab2in = din("ab2in", [1, BL], F32)
    onescol = din("onescol", [P, 1])
    onesrow = din("onesrow", [1, P])
    mkt = nc.dram_tensor("mkt", [BL, C, L], F32, kind="ExternalOutput")
    mvt = nc.dram_tensor("mvt", [BL, C, L], F32, kind="ExternalOutput")

    def r3(ap):  # [X*P, N] dram -> [P, X, N]
        return ap.rearrange("(o p) n -> p o n", p=P)

    with tile.TileContext(nc) as tc:
        with (
            tc.tile_pool(name="const", bufs=1) as constp,
            tc.tile_pool(name="persist", bufs=1) as pers,
            tc.tile_pool(name="work", bufs=1) as work,
            tc.tile_pool(name="evict", bufs=2) as evp,
            tc.tile_pool(name="evq", bufs=1) as evq,
            tc.tile_pool(name="psum", bufs=3, space="PSUM") as psp,
            tc.tile_pool(name="psmall", bufs=1, space="PSUM") as pss,
        ):
            ones_col = constp.tile([P, 1], R32)
            nc.sync.dma_start(ones_col[:], onescol[:])
            ones_row = constp.tile([1, P], R32)
            nc.sync.dma_start(ones_row[:], onesrow[:])

            # ---------- phase 0: skT / sv (B-independent) ----------
            # Wk/Wv fp32r streamed in 128-row chunks; k-partials accumulate
            # in SBUF (PSUM can't hold 8 concurrent groups alongside pools).
            skt = pers.tile([P, CO, VS], R32)
            sv = pers.tile([P, 3, C], R32)
            with (
                tc.tile_pool(name="ph0s", bufs=1) as ph0s,
                tc.tile_pool(name="ph0w", bufs=1) as ph0,
            ):
                smt_sb = ph0s.tile([P, CO, VS], R32, tag="smt")
                nc.sync.dma_start(smt_sb[:], r3(smt))
                for k in range(CO):
                    wkch = ph0.tile([P, C], R32, tag="wch")
                    nc.sync.dma_start(wkch[:], wk[k * P:(k + 1) * P, :])
                    wvch = ph0.tile([P, C], R32, tag="wch")
                    nc.sync.dma_start(wvch[:], wv[k * P:(k + 1) * P, :])
                    for m in range(CO):
                        ps = psp.tile([P, VS], F32, tag="ps")
                        nc.tensor.matmul(ps[:], wkch[:, bass.ts(m, P)],
                                         smt_sb[:, k, :],
                                         start=True, stop=True)
                        if k == 0:
                            nc.vector.tensor_copy(skt[:, m, :], ps[:])
                        else:
                            nc.vector.tensor_tensor(
                                skt[:, m, :], skt[:, m, :], ps[:],
                                mybir.AluOpType.add)
                    for mc, (vs0, rows) in enumerate(VS_CHUNKS):
                        for n in range(2):
                            ps = psp.tile([P, 512], F32, tag="ps")
                            nc.tensor.matmul(
                                ps[:rows], smt_sb[:, k, vs0:vs0 + rows],
                                wvch[:, bass.ts(n, 512)],
                                start=True, stop=True)
                            if k == 0:
                                nc.vector.tensor_copy(
                                    sv[:rows, mc, bass.ts(n, 512)], ps[:rows])
                            else:
                                nc.vector.tensor_tensor(
                                    sv[:rows, mc, bass.ts(n, 512)],
                                    sv[:rows, mc, bass.ts(n, 512)], ps[:rows],
                                    mybir.AluOpType.add)

            # ---------- phase 1: deltaT (all b), bf16 ----------
            deltat = pers.tile([P, CO, BL * VS], BF16)
            with (
                tc.tile_pool(name="ph1", bufs=4) as ph1,
                tc.tile_pool(name="ph1ps", bufs=2, space="PSUM") as ph1ps,
            ):
                abd_sb = ph1.tile([S * R, BL * S], BF16, tag="abd")
                nc.sync.dma_start(abd_sb[:], abd[:])
                for s in range(SLOTS):
                    catb_s = ph1.tile([S * R, C], BF16, tag="catb_s")
                    nc.sync.dma_start(catb_s[:], catb[:, s * C:(s + 1) * C])
                    for h in range(2):
                        psd = ph1ps.tile([P, 4, P], F32, tag="ps_delta")
                        for j in range(4):
                            o = 4 * h + j
                            nc.tensor.matmul(
                                psd[:, j, :BL * S],
                                catb_s[:, o * P:(o + 1) * P],
                                abd_sb[:], start=True, stop=True)
                        # scatter (b,v) cols to b*320 + v*32 + s == s::32
                        nc.vector.tensor_copy(
                            deltat[:, 4 * h:4 * h + 4, s::SLOTS],
                            psd[:, :, :BL * S])

            # ---------- persistent weights ----------
            wq_sb = pers.tile([P, CO, C], R32)
            nc.sync.dma_start(wq_sb[:], r3(wq))
            wkb_sb = pers.tile([P, CO, C], BF16)
            nc.sync.dma_start(wkb_sb[:], r3(wkb))
            wvb_sb = pers.tile([P, CO, C], BF16)
            nc.sync.dma_start(wvb_sb[:], r3(wvb))
            aw1x_sb = pers.tile([P, CO, AH], R32)
            nc.sync.dma_start(aw1x_sb[:], r3(aw1x))
            aw2_sb = pers.tile([P, 1], R32)
            nc.sync.dma_start(aw2_sb[:], aw2[:])
            bias_sb = pers.tile([P, BL, S], F32)
            nc.sync.dma_start(bias_sb[:], biasall[:])
            kw1v1_sb = pers.tile([P, CO, 128], R32)
            nc.sync.dma_start(kw1v1_sb[:], r3(kw1v1))
            kb1v1_sb = pers.tile([P, 1], F32)
            nc.sync.dma_start(kb1v1_sb[:], kb1v1[:])
            kw2v2_sb = pers.tile([P, C], R32)
            nc.sync.dma_start(kw2v2_sb[:], kw2v2[:])
            kb2s_sb = pers.tile([P, CO], F32)
            nc.sync.dma_start(kb2s_sb[:], kb2s[:])
            vb2s_sb = pers.tile([P, CO], F32)
            nc.sync.dma_start(vb2s_sb[:], vb2s[:])
            ab2_sb = pers.tile([1, BL], F32)
            nc.sync.dma_start(ab2_sb[:], ab2in[:])

            # visibility mask [P, 3, L]: 1 where key-row valid for the scale
            # owning column l, else 0. bf16 (values exact).
            mask = pers.tile([P, 3, L], BF16)
            nc.vector.memset(mask[:], 0.0)
            for i, (s0, e0) in enumerate(BEGIN_ENDS):
                nvis = SLOTS * (i + 1)
                for mc, (vs0, rows) in enumerate(VS_CHUNKS):
                    vr = min(rows, nvis - vs0)
                    if vr > 0:
                        nc.vector.memset(mask[:vr, mc, s0:e0], 1.0)

            # ---------- per-b main loop ----------
            for b in range(BL):
                # kcT = skT + Wk^T @ deltaT_b   (bf16 matmul, add at evict)
                kct = work.tile([P, CO, VS], R32, tag="kct")
                for m in range(CO):
                    ps = psp.tile([P, VS], F32, tag="ps")
                    for k in range(CO):
                        nc.tensor.matmul(
                            ps[:], wkb_sb[:, k, bass.ts(m, P)],
                            deltat[:, k, b * VS:(b + 1) * VS],
                            start=(k == 0), stop=(k == CO - 1))
                    nc.vector.tensor_tensor(kct[:, m, :], ps[:], skt[:, m, :],
                                            mybir.AluOpType.add)
                # vc = sv + deltaT_b^T @ Wv
                vct = work.tile([P, 3, C], R32, tag="vct")
                for mc, (vs0, rows) in enumerate(VS_CHUNKS):
                    for n in range(2):
                        ps = psp.tile([P, 512], F32, tag="ps")
                        for k in range(CO):
                            nc.tensor.matmul(
                                ps[:rows],
                                deltat[:, k, b * VS + vs0: b * VS + vs0 + rows],
                                wvb_sb[:, k, bass.ts(n, 512)],
                                start=(k == 0), stop=(k == CO - 1))
                        nc.vector.tensor_tensor(
                            vct[:rows, mc, bass.ts(n, 512)], ps[:rows],
                            sv[:rows, mc, bass.ts(n, 512)],
                            mybir.AluOpType.add)

                # ------- per L-half (l0:l0+lw) -------
                for n0, lw in NT:
                    # xT slice load (tag shared with memT: disjoint lifetimes)
                    xt_sb = work.tile([P, CO, 340], R32, tag="bigCL_a")
                    nc.sync.dma_start(xt_sb[:, :, :lw],
                                      r3(xt[b])[:, :, n0:n0 + lw])
                    # qT = Wq^T @ xT
                    qt = work.tile([P, CO, 340], R32, tag="qt")
                    for m in range(CO):
                        ps = psp.tile([P, 340], F32, tag="ps")
                        for k in range(CO):
                            nc.tensor.matmul(
                                ps[:, :lw], wq_sb[:, k, bass.ts(m, P)],
                                xt_sb[:, k, :lw],
                                start=(k == 0), stop=(k == CO - 1))
                        nc.vector.tensor_copy(qt[:, m, :lw], ps[:, :lw])

                    # alpha = sigmoid(aW2^T gelu(aW1x^T qT + bias) + ab2)
                    g = evq.tile([P, 340], R32, tag="gelu")
                    psa = psp.tile([P, 340], F32, tag="ps")
                    for k in range(CO):
                        nc.tensor.matmul(psa[:, :lw], aw1x_sb[:, k, :],
                                         qt[:, k, :lw],
                                         start=(k == 0), stop=(k == CO - 1))
                    for i, (s0, e0) in enumerate(BEGIN_ENDS):
                        c0, c1 = max(s0, n0), min(e0, n0 + lw)
                        if c0 < c1:
                            nc.scalar.activation(
                                g[:, c0 - n0:c1 - n0],
                                psa[:, c0 - n0:c1 - n0],
                                mybir.ActivationFunctionType.Gelu,
                                bias=bias_sb[:, b, i:i + 1])
                    alpha = constp.tile([1, 340], F32, tag="alpha")
                    psz = pss.tile([1, 340], F32, tag="ps_zb")
                    nc.tensor.matmul(psz[:, :lw], aw2_sb[:], g[:, :lw],
                                     start=True, stop=True)
                    nc.scalar.activation(alpha[:, :lw], psz[:, :lw],
                                         mybir.ActivationFunctionType.Sigmoid,
                                         bias=ab2_sb[0:1, b:b + 1])

                    # ---- attention scores/exp for both paths ----
                    def scores_exp(keys, out_tag, n0=n0, lw=lw, qt=qt):
                        ex = work.tile([P, 3, 340], R32, tag=out_tag)
                        for mc, (vs0, rows) in enumerate(VS_CHUNKS):
                            ps = psp.tile([P, 340], F32, tag="ps")
                            for k in range(CO):
                                nc.tensor.matmul(
                                    ps[:rows, :lw],
                                    keys[:, k, vs0:vs0 + rows],
                                    qt[:, k, :lw],
                                    start=(k == 0), stop=(k == CO - 1))
                            nc.scalar.activation(
                                ex[:rows, mc, :lw], ps[:rows, :lw],
                                mybir.ActivationFunctionType.Exp,
                                scale=sc_scale)
                        # visibility mask
                        nc.vector.tensor_tensor(
                            ex[:, :, :lw], ex[:, :, :lw],
                            mask[:, :, n0:n0 + lw], mybir.AluOpType.mult)
                        return ex

                    def col_sums(ex, tag, lw=lw):
                        sums = pss.tile([1, 340], F32, tag="ps_sum_" + tag)
                        for mc, (vs0, rows) in enumerate(VS_CHUNKS):
                            nc.tensor.matmul(
                                sums[:, :lw], ones_col[:rows],
                                ex[:rows, mc, :lw],
                                start=(mc == 0), stop=(mc == 2))
                        return sums

                    exs = scores_exp(skt, "exp_s")
                    sums_s = col_sums(exs, "s")
                    exc = scores_exp(kct, "exp_c")
                    sums_c = col_sums(exc, "c")

                    # f1 = (1-alpha)/S_s, f2 = alpha/S_c ; broadcast to [P, .]
                    f12 = constp.tile([1, 2, 340], R32, tag="f12")
                    rec = constp.tile([1, 2, 340], F32, tag="rec")
                    nc.vector.reciprocal(rec[0:1, 0, :lw], sums_s[:, :lw])
                    nc.vector.reciprocal(rec[0:1, 1, :lw], sums_c[:, :lw])
                    one_minus = constp.tile([1, 340], F32, tag="onem")
                    nc.vector.tensor_scalar(one_minus[:, :lw], alpha[:, :lw],
                                            -1.0, 1.0,
                                            mybir.AluOpType.mult,
                                            mybir.AluOpType.add)
                    nc.vector.tensor_tensor(f12[0:1, 0, :lw],
                                            one_minus[:, :lw],
                                            rec[0:1, 0, :lw],
                                            mybir.AluOpType.mult)
                    nc.vector.tensor_tensor(f12[0:1, 1, :lw], alpha[:, :lw],
                                            rec[0:1, 1, :lw],
                                            mybir.AluOpType.mult)
                    fb = evq.tile([P, 2, 340], R32, tag="fbcast")
                    for j in range(2):
                        psb = pss.tile([P, 340], F32, tag="ps_zb")
                        nc.tensor.matmul(psb[:, :lw], ones_row[:],
                                         f12[0:1, j, :lw],
                                         start=True, stop=True)
                        nc.vector.tensor_copy(fb[:, j, :lw], psb[:, :lw])
                    for mc, (vs0, rows) in enumerate(VS_CHUNKS):
                        nc.vector.tensor_tensor(
                            exs[:rows, mc, :lw], exs[:rows, mc, :lw],
                            fb[:rows, 0, :lw], mybir.AluOpType.mult)
                        nc.vector.tensor_tensor(
                            exc[:rows, mc, :lw], exc[:rows, mc, :lw],
                            fb[:rows, 1, :lw], mybir.AluOpType.mult)

                    # memT = sv^T @ exp_s + vc^T @ exp_c (one PSUM group)
                    memt = work.tile([P, CO, 340], R32, tag="bigCL_a")
                    for o in range(CO):
                        ps = psp.tile([P, 340], F32, tag="ps")
                        for mc, (vs0, rows) in enumerate(VS_CHUNKS):
                            nc.tensor.matmul(
                                ps[:, :lw], sv[:rows, mc, bass.ts(o, P)],
                                exs[:rows, mc, :lw],
                                start=(mc == 0), stop=False)
                        for mc, (vs0, rows) in enumerate(VS_CHUNKS):
                            nc.tensor.matmul(
                                ps[:, :lw], vct[:rows, mc, bass.ts(o, P)],
                                exc[:rows, mc, :lw],
                                start=False, stop=(mc == 2))
                        nc.vector.tensor_copy(memt[:, o, :lw], ps[:, :lw])

                    # final projections
                    mk1 = evq.tile([P, 340], R32, tag="mk1")
                    ps1 = psp.tile([P, 340], F32, tag="ps")
                    for k in range(CO):
                        nc.tensor.matmul(ps1[:, :lw], kw1v1_sb[:, k, :],
                                         memt[:, k, :lw],
                                         start=(k == 0), stop=(k == CO - 1))
                    nc.scalar.activation(mk1[:, :lw], ps1[:, :lw],
                                         mybir.ActivationFunctionType.Identity,
                                         bias=kb1v1_sb[:])
                    for o in range(CO):
                        psk = psp.tile([P, 340], F32, tag="ps")
                        nc.tensor.matmul(psk[:, :lw],
                                         kw2v2_sb[0:64, bass.ts(o, P)],
                                         mk1[0:64, :lw],
                                         start=True, stop=True)
                        ok = evp.tile([P, 340], F32, tag="outev")
                        nc.scalar.activation(
                            ok[:, :lw], psk[:, :lw],
                            mybir.ActivationFunctionType.Identity,
                            bias=kb2s_sb[:, o:o + 1])
                        nc.sync.dma_start(
                            r3(mkt[b])[:, o, n0:n0 + lw], ok[:, :lw])
                        psv = psp.tile([P, 340], F32, tag="ps")
                        nc.tensor.matmul(psv[:, :lw],
                                         kw2v2_sb[64:128, bass.ts(o, P)],
                                         mk1[64:128, :lw],
                                         start=True, stop=True)
                        ov = evp.tile([P, 340], F32, tag="outev")
                        nc.scalar.activation(
                            ov[:, :lw], psv[:, :lw],
                            mybir.ActivationFunctionType.Identity,
                            bias=vb2s_sb[:, o:o + 1])
                        nc.sync.dma_start(
                            r3(mvt[b])[:, o, n0:n0 + lw], ov[:, :lw])
    nc.compile()
    return nc


def kernel(x, category_ids, shared_memory, cat_A, cat_B, cat_emb, scale_emb,
           Wq, Wk, Wv, aW1, ab1, aW2, ab2,
           kW1, kb1, kW2, kb2, vW1, vb1, vW2, vb2,
           gk_logit, gv_logit, log_temp):
    f32 = np.float32
    x = np.asarray(x, f32)
    category_ids = np.asarray(category_ids)
    shared_memory = np.asarray(shared_memory, f32)
    cat_A = np.asarray(cat_A, f32)
    cat_B = np.asarray(cat_B, f32)
    cat_emb = np.asarray(cat_emb, f32)
    scale_emb = np.asarray(scale_emb, f32)
    Wq, Wk, Wv = (np.asarray(a, f32) for a in (Wq, Wk, Wv))
    aW1, ab1 = np.asarray(aW1, f32), np.asarray(ab1, f32)
    aW2, ab2 = np.asarray(aW2, f32), np.asarray(ab2, f32)
    kW1, kb1, kW2, kb2 = (np.asarray(a, f32) for a in (kW1, kb1, kW2, kb2))
    vW1, vb1, vW2, vb2 = (np.asarray(a, f32) for a in (vW1, vb1, vW2, vb2))

    temp = float(np.clip(np.exp(np.asarray(log_temp, f32)), 0.05, 1.0))
    sc_scale = float((1.0 / math.sqrt(C)) / temp)
    sig_gk = float(1.0 / (1.0 + np.exp(-np.asarray(gk_logit, f32))))
    sig_gv = float(1.0 / (1.0 + np.exp(-np.asarray(gv_logit, f32))))

    valid = (category_ids >= 0)
    cid = np.clip(category_ids, 0, None).astype(np.int64)
    a_all = cat_A[cid]  # [B, S, R]
    ce_alpha = cat_emb[cid] @ aW1[C:2 * C]  # [B, AH]
    se_alpha = scale_emb @ aW1[2 * C:3 * C]  # [S, AH]

    bf16 = ml_dtypes.bfloat16
    smt = np.ascontiguousarray(shared_memory.reshape(VS, C).T)
    catb = np.ascontiguousarray(cat_B.reshape(S * R, SLOTS * C)).astype(bf16)
    kw1v1 = np.ascontiguousarray(np.concatenate([kW1, vW1], axis=1))
    kb1v1 = np.concatenate([kb1, vb1]).reshape(128, 1).astype(f32)
    kw2v2 = np.ascontiguousarray(
        np.concatenate([sig_gk * kW2, sig_gv * vW2], axis=0))
    kb2s = np.ascontiguousarray(
        (sig_gk * kb2).reshape(CO, P).T).astype(f32)  # [P, CO]
    vb2s = np.ascontiguousarray((sig_gv * vb2).reshape(CO, P).T).astype(f32)

    # per-core tensors
    in_maps = []
    ab2_all = []
    for core in range(NCORES):
        bs = slice(core * BL, (core + 1) * BL)
        xt = np.ascontiguousarray(x[bs].transpose(0, 2, 1))
        abd = np.zeros((S * R, BL * S), dtype=f32)
        for b in range(BL):
            for v in range(S):
                abd[v * R:(v + 1) * R, b * S + v] = a_all[bs][b, v]
        biasall = (ce_alpha[bs][:, None, :] + se_alpha[None, :, :]
                   + ab1[None, None, :])  # [BL, S, AH]
        biasall = np.ascontiguousarray(biasall.transpose(2, 0, 1)).astype(f32)
        ab2_b = tuple(float(ab2[0]) if valid[bs][b] else -30.0
                      for b in range(BL))
        ab2_all.append(ab2_b)
        in_maps.append({
            "xt": xt, "smt": smt, "wq": Wq, "wk": Wk, "wv": Wv,
            "wkb": Wk.astype(bf16), "wvb": Wv.astype(bf16),
            "catb": catb, "abd": abd.astype(bf16),
            "aw1x": np.ascontiguousarray(aW1[:C]), "aw2": aW2,
            "biasall": biasall, "kw1v1": kw1v1, "kb1v1": kb1v1,
            "kw2v2": kw2v2, "kb2s": kb2s, "vb2s": vb2s,
            "ab2in": np.array([ab2_b], dtype=f32),
            "onescol": np.ones((128, 1), f32),
            "onesrow": np.ones((1, 128), f32),
        })

    global _last_in_maps
    _last_in_maps = in_maps
    key = sc_scale
    if key not in _nc_cache:
        _nc_cache.clear()
        _nc_cache[key] = _build(sc_scale)
    ncs = _nc_cache[key]
    res = run_bass_kernel_spmd(ncs, in_maps, core_ids=list(range(NCORES)))

    mem_k = np.empty((B, L, C), f32)
    mem_v = np.empty((B, L, C), f32)
    for core in range(NCORES):
        r = res.results[core]
        bs = slice(core * BL, (core + 1) * BL)
        mem_k[bs] = r["mkt"].transpose(0, 2, 1)
        mem_v[bs] = r["mvt"].transpose(0, 2, 1)
    zero = np.float32(0.0)
    return mem_k, mem_v, zero, zero


# revision 23
# speedup vs baseline: 1.1553x; 1.1553x over previous
"""ClassAwareKnittingMemoryV2 Trainium2 kernel (8 NeuronCores, data-parallel over B).

Math restructure vs the reference:
  - query/attention computed fully "transposed" (feature-major [C, L]); host
    transposes x (and transposes back the outputs), so the device needs no
    transposes at all.
  - cat_mem = shared_memory + delta, so per-category keys/values split as
      kcT = skT + Wk^T @ deltaT,   vc = sv + deltaT^T @ Wv
    with skT/sv computed once (B-independent) and deltaT computed c-major
    directly via 256 tiny bf16 matmuls:
      deltaT[c_win, (b,v)] = (cat_B window [80, 128])^T @ a_blockdiag [80, 40]
    (windows are pure-c because cat_B's free dim is (s, c) with c inner).
  - Scores are computed transposed ([keys, L]); softmax runs over the
    partition dim: exp (no max subtraction -- scaled scores are within +/-3),
    per-scale visibility handled by a 0/1 mask multiply, sums via ones-matmul,
    and the (1-alpha)/sum_s resp. alpha/sum_c normalization factors are
    broadcast to 128 partitions with a K=1 ones-matmul and multiplied into the
    exp tiles.  Both attention outputs then accumulate into one PSUM group:
      memT = sv^T @ (exp_s * f1) + vc^T @ (exp_c * f2),  f1=(1-a)/S_s, f2=a/S_c
  - alpha MLP: aW1 splits into x/ce/scale parts; ce/scale parts and all the
    tiny gathers (cat_A[cid], cat_emb[cid]) are host-side; sigmoid gate of an
    invalid (CFG) category is forced ~0 by biasing ab2 to -30.
  - sigmoid(gk/gv) gates and temp are folded into host-prepared weights.

Matmul dtype: float32r (full PE rate at N>=256, ~1.5e-4 rel err) everywhere
except the delta pipeline (bf16; delta is ~30x smaller than shared_memory so
bf16 there contributes ~1e-4 relative).
"""

import math

import numpy as np

import concourse.bass as bass
import concourse.tile as tile
from concourse import bacc, mybir
from concourse.bass_utils import run_bass_kernel_spmd

# ---- problem constants (hardcoded per contract) ----
B, C, L = 32, 1024, 680
S, SLOTS, R, NCAT = 10, 32, 8, 22
VS = S * SLOTS  # 320
MEM_RANK, AH = 64, 128
PATCH_NUMS = (1, 2, 3, 4, 5, 6, 8, 10, 13, 16)
_lens = [p * p for p in PATCH_NUMS]
_cum = np.concatenate([[0], np.cumsum(_lens)])
BEGIN_ENDS = [(int(_cum[i]), int(_cum[i + 1])) for i in range(S)]

NCORES = 8
BL = B // NCORES  # 4 batch elements per core
P = 128
CO = C // P  # 8 c-chunks
NT = [(0, 340), (340, 340)]  # (start, width) l n-tiles (>=256: fp32r full rate)
VS_CHUNKS = [(0, 128), (128, 128), (256, 64)]  # (start, rows) of key chunks

F32 = mybir.dt.float32
R32 = mybir.dt.float32r
BF16 = mybir.dt.bfloat16
MD = BF16  # matmul operand dtype (BF16 or R32)

import ml_dtypes

_nc_cache = {}
_last_in_maps = None


def _build(sc_scale: float) -> bass.Bass:
    nc = bacc.Bacc("TRN2", target_bir_lowering=False, debug=False,
                   num_devices=NCORES)

    def din(name, shape, dt=MD):
        return nc.dram_tensor(name, shape, dt, kind="ExternalInput")

    xt = din("xt", [BL, C, L])
    smt = din("smt", [C, VS])
    wq = din("wq", [C, C])
    wk = din("wk", [C, C])
    wv = din("wv", [C, C])
    wkb = din("wkb", [C, C], BF16)
    wvb = din("wvb", [C, C], BF16)
    catb = din("catb", [S * R, SLOTS * C], BF16)
    abd = din("abd", [S * R, BL * S], BF16)
    aw1x = din("aw1x", [C, AH])
    aw2 = din("aw2", [AH, 1])
    biasall = din("biasall", [AH, BL, S], F32)
    kw1v1 = din("kw1v1", [C, 128])
    kb1v1 = din("kb1v1", [128, 1], F32)
    kw2v2 = din("kw2v2", [128, C])
    kb2s = din("kb2s", [P, CO], F32)
    vb2s = din("vb2s", [P, CO], F32)
    ab2in = din("ab2in", [1, BL], F32)
    onescol = din("onescol", [P, 1])
    onesrow = din("onesrow", [1, P])
    mkt = nc.dram_tensor("mkt", [BL, C, L], F32, kind="ExternalOutput")
    mvt = nc.dram_tensor("mvt", [BL, C, L], F32, kind="ExternalOutput")

    def r3(ap):  # [X*P, N] dram -> [P, X, N]
        return ap.rearrange("(o p) n -> p o n", p=P)

    with tile.TileContext(nc) as tc:
        with (
            tc.tile_pool(name="const", bufs=1) as constp,
            tc.tile_pool(name="persist", bufs=1) as pers,
            tc.tile_pool(name="work", bufs=1) as work,
            tc.tile_pool(name="evict", bufs=2) as evp,
            tc.tile_pool(name="evq", bufs=1) as evq,
            tc.tile_pool(name="psum", bufs=3, space="PSUM") as psp,
            tc.tile_pool(name="psmall", bufs=1, space="PSUM") as pss,
        ):
            ones_col = constp.tile([P, 1], MD)
            nc.sync.dma_start(ones_col[:], onescol[:])
            ones_row = constp.tile([1, P], MD)
            nc.sync.dma_start(ones_row[:], onesrow[:])

            # ---------- phase 0: skT / sv (B-independent) ----------
            # Wk/Wv fp32r streamed in 128-row chunks; k-partials accumulate
            # in SBUF (PSUM can't hold 8 concurrent groups alongside pools).
            skt = pers.tile([P, CO, VS], MD)
            sv = pers.tile([P, 3, C], MD)
            with (
                tc.tile_pool(name="ph0s", bufs=1) as ph0s,
                tc.tile_pool(name="ph0w", bufs=1) as ph0,
            ):
                smt_sb = ph0s.tile([P, CO, VS], MD, tag="smt")
                nc.sync.dma_start(smt_sb[:], r3(smt))
                for k in range(CO):
                    wkch = ph0.tile([P, C], MD, tag="wch")
                    nc.sync.dma_start(wkch[:], wk[k * P:(k + 1) * P, :])
                    wvch = ph0.tile([P, C], MD, tag="wch")
                    nc.sync.dma_start(wvch[:], wv[k * P:(k + 1) * P, :])
                    for m in range(CO):
                        ps = psp.tile([P, VS], F32, tag="ps")
                        nc.tensor.matmul(ps[:], wkch[:, bass.ts(m, P)],
                                         smt_sb[:, k, :],
                                         start=True, stop=True)
                        if k == 0:
                            nc.vector.tensor_copy(skt[:, m, :], ps[:])
                        else:
                            nc.vector.tensor_tensor(
                                skt[:, m, :], skt[:, m, :], ps[:],
                                mybir.AluOpType.add)
                    for mc, (vs0, rows) in enumerate(VS_CHUNKS):
                        for n in range(2):
                            ps = psp.tile([P, 512], F32, tag="ps")
                            nc.tensor.matmul(
                                ps[:rows], smt_sb[:, k, vs0:vs0 + rows],
                                wvch[:, bass.ts(n, 512)],
                                start=True, stop=True)
                            if k == 0:
                                nc.vector.tensor_copy(
                                    sv[:rows, mc, bass.ts(n, 512)], ps[:rows])
                            else:
                                nc.vector.tensor_tensor(
                                    sv[:rows, mc, bass.ts(n, 512)],
                                    sv[:rows, mc, bass.ts(n, 512)], ps[:rows],
                                    mybir.AluOpType.add)

            # ---------- phase 1: deltaT (all b), bf16 ----------
            deltat = pers.tile([P, CO, BL * VS], BF16)
            with (
                tc.tile_pool(name="ph1", bufs=4) as ph1,
                tc.tile_pool(name="ph1ps", bufs=2, space="PSUM") as ph1ps,
            ):
                abd_sb = ph1.tile([S * R, BL * S], BF16, tag="abd")
                nc.sync.dma_start(abd_sb[:], abd[:])
                for s in range(SLOTS):
                    catb_s = ph1.tile([S * R, C], BF16, tag="catb_s")
                    nc.sync.dma_start(catb_s[:], catb[:, s * C:(s + 1) * C])
                    for h in range(2):
                        psd = ph1ps.tile([P, 4, P], F32, tag="ps_delta")
                        for j in range(4):
                            o = 4 * h + j
                            nc.tensor.matmul(
                                psd[:, j, :BL * S],
                                catb_s[:, o * P:(o + 1) * P],
                                abd_sb[:], start=True, stop=True)
                        # scatter (b,v) cols to b*320 + v*32 + s == s::32
                        nc.vector.tensor_copy(
                            deltat[:, 4 * h:4 * h + 4, s::SLOTS],
                            psd[:, :, :BL * S])

            # ---------- persistent weights ----------
            wq_sb = pers.tile([P, CO, C], MD)
            nc.sync.dma_start(wq_sb[:], r3(wq))
            wkb_sb = pers.tile([P, CO, C], BF16)
            nc.sync.dma_start(wkb_sb[:], r3(wkb))
            wvb_sb = pers.tile([P, CO, C], BF16)
            nc.sync.dma_start(wvb_sb[:], r3(wvb))
            aw1x_sb = pers.tile([P, CO, AH], MD)
            nc.sync.dma_start(aw1x_sb[:], r3(aw1x))
            aw2_sb = pers.tile([P, 1], MD)
            nc.sync.dma_start(aw2_sb[:], aw2[:])
            bias_sb = pers.tile([P, BL, S], F32)
            nc.sync.dma_start(bias_sb[:], biasall[:])
            kw1v1_sb = pers.tile([P, CO, 128], MD)
            nc.sync.dma_start(kw1v1_sb[:], r3(kw1v1))
            kb1v1_sb = pers.tile([P, 1], F32)
            nc.sync.dma_start(kb1v1_sb[:], kb1v1[:])
            kw2v2_sb = pers.tile([P, C], MD)
            nc.sync.dma_start(kw2v2_sb[:], kw2v2[:])
            kb2s_sb = pers.tile([P, CO], F32)
            nc.sync.dma_start(kb2s_sb[:], kb2s[:])
            vb2s_sb = pers.tile([P, CO], F32)
            nc.sync.dma_start(vb2s_sb[:], vb2s[:])
            ab2_sb = pers.tile([1, BL], F32)
            nc.sync.dma_start(ab2_sb[:], ab2in[:])

            # visibility mask [P, 3, L]: 1 where key-row valid for the scale
            # owning column l, else 0. bf16 (values exact).
            mask = pers.tile([P, 3, L], BF16)
            nc.vector.memset(mask[:], 0.0)
            for i, (s0, e0) in enumerate(BEGIN_ENDS):
                nvis = SLOTS * (i + 1)
                for mc, (vs0, rows) in enumerate(VS_CHUNKS):
                    vr = min(rows, nvis - vs0)
                    if vr > 0:
                        nc.vector.memset(mask[:vr, mc, s0:e0], 1.0)

            # ---------- per-b main loop ----------
            for b in range(BL):
                # kcT = skT + Wk^T @ deltaT_b   (bf16 matmul, add at evict)
                kct = work.tile([P, CO, VS], MD, tag="kct")
                for m in range(CO):
                    ps = psp.tile([P, VS], F32, tag="ps")
                    for k in range(CO):
                        nc.tensor.matmul(
                            ps[:], wkb_sb[:, k, bass.ts(m, P)],
                            deltat[:, k, b * VS:(b + 1) * VS],
                            start=(k == 0), stop=(k == CO - 1))
                    nc.vector.tensor_tensor(kct[:, m, :], ps[:], skt[:, m, :],
                                            mybir.AluOpType.add)
                # vc = sv + deltaT_b^T @ Wv
                vct = work.tile([P, 3, C], MD, tag="vct")
                for mc, (vs0, rows) in enumerate(VS_CHUNKS):
                    for n in range(2):
                        ps = psp.tile([P, 512], F32, tag="ps")
                        for k in range(CO):
                            nc.tensor.matmul(
                                ps[:rows],
                                deltat[:, k, b * VS + vs0: b * VS + vs0 + rows],
                                wvb_sb[:, k, bass.ts(n, 512)],
                                start=(k == 0), stop=(k == CO - 1))
                        nc.vector.tensor_tensor(
                            vct[:rows, mc, bass.ts(n, 512)], ps[:rows],
                            sv[:rows, mc, bass.ts(n, 512)],
                            mybir.AluOpType.add)

                # ------- per L-half (l0:l0+lw) -------
                for n0, lw in NT:
                    # xT slice load (tag shared with memT: disjoint lifetimes)
                    xt_sb = work.tile([P, CO, 340], MD, tag="bigCL_a")
                    nc.sync.dma_start(xt_sb[:, :, :lw],
                                      r3(xt[b])[:, :, n0:n0 + lw])
                    # qT = Wq^T @ xT
                    qt = work.tile([P, CO, 340], MD, tag="qt")
                    for m in range(CO):
                        ps = psp.tile([P, 340], F32, tag="ps")
                        for k in range(CO):
                            nc.tensor.matmul(
                                ps[:, :lw], wq_sb[:, k, bass.ts(m, P)],
                                xt_sb[:, k, :lw],
                                start=(k == 0), stop=(k == CO - 1))
                        nc.vector.tensor_copy(qt[:, m, :lw], ps[:, :lw])

                    # alpha = sigmoid(aW2^T gelu(aW1x^T qT + bias) + ab2)
                    g = evq.tile([P, 340], MD, tag="gelu")
                    psa = psp.tile([P, 340], F32, tag="ps")
                    for k in range(CO):
                        nc.tensor.matmul(psa[:, :lw], aw1x_sb[:, k, :],
                                         qt[:, k, :lw],
                                         start=(k == 0), stop=(k == CO - 1))
                    for i, (s0, e0) in enumerate(BEGIN_ENDS):
                        c0, c1 = max(s0, n0), min(e0, n0 + lw)
                        if c0 < c1:
                            nc.scalar.activation(
                                g[:, c0 - n0:c1 - n0],
                                psa[:, c0 - n0:c1 - n0],
                                mybir.ActivationFunctionType.Gelu,
                                bias=bias_sb[:, b, i:i + 1])
                    alpha = constp.tile([1, 340], F32, tag="alpha")
                    psz = pss.tile([1, 340], F32, tag="ps_zb")
                    nc.tensor.matmul(psz[:, :lw], aw2_sb[:], g[:, :lw],
                                     start=True, stop=True)
                    nc.scalar.activation(alpha[:, :lw], psz[:, :lw],
                                         mybir.ActivationFunctionType.Sigmoid,
                                         bias=ab2_sb[0:1, b:b + 1])

                    # ---- attention scores/exp for both paths ----
                    def scores_exp(keys, out_tag, n0=n0, lw=lw, qt=qt):
                        ex = work.tile([P, 3, 340], MD, tag=out_tag)
                        for mc, (vs0, rows) in enumerate(VS_CHUNKS):
                            ps = psp.tile([P, 340], F32, tag="ps")
                            for k in range(CO):
                                nc.tensor.matmul(
                                    ps[:rows, :lw],
                                    keys[:, k, vs0:vs0 + rows],
                                    qt[:, k, :lw],
                                    start=(k == 0), stop=(k == CO - 1))
                            nc.scalar.activation(
                                ex[:rows, mc, :lw], ps[:rows, :lw],
                                mybir.ActivationFunctionType.Exp,
                                scale=sc_scale)
                        # visibility mask
                        nc.vector.tensor_tensor(
                            ex[:, :, :lw], ex[:, :, :lw],
                            mask[:, :, n0:n0 + lw], mybir.AluOpType.mult)
                        return ex

                    def col_sums(ex, tag, lw=lw):
                        sums = pss.tile([1, 340], F32, tag="ps_sum_" + tag)
                        for mc, (vs0, rows) in enumerate(VS_CHUNKS):
                            nc.tensor.matmul(
                                sums[:, :lw], ones_col[:rows],
                                ex[:rows, mc, :lw],
                                start=(mc == 0), stop=(mc == 2))
                        return sums

                    exs = scores_exp(skt, "exp_s")
                    sums_s = col_sums(exs, "s")
                    exc = scores_exp(kct, "exp_c")
                    sums_c = col_sums(exc, "c")

                    # f1 = (1-alpha)/S_s, f2 = alpha/S_c ; broadcast to [P, .]
                    f12 = constp.tile([1, 2, 340], MD, tag="f12")
                    rec = constp.tile([1, 2, 340], F32, tag="rec")
                    nc.vector.reciprocal(rec[0:1, 0, :lw], sums_s[:, :lw])
                    nc.vector.reciprocal(rec[0:1, 1, :lw], sums_c[:, :lw])
                    one_minus = constp.tile([1, 340], F32, tag="onem")
                    nc.vector.tensor_scalar(one_minus[:, :lw], alpha[:, :lw],
                                            -1.0, 1.0,
                                            mybir.AluOpType.mult,
                                            mybir.AluOpType.add)
                    nc.vector.tensor_tensor(f12[0:1, 0, :lw],
                                            one_minus[:, :lw],
                                            rec[0:1, 0, :lw],
                                            mybir.AluOpType.mult)
                    nc.vector.tensor_tensor(f12[0:1, 1, :lw], alpha[:, :lw],
                                            rec[0:1, 1, :lw],
                                            mybir.AluOpType.mult)
                    fb = evq.tile([P, 2, 340], MD, tag="fbcast")
                    for j in range(2):
                        psb = pss.tile([P, 340], F32, tag="ps_zb")
                        nc.tensor.matmul(psb[:, :lw], ones_row[:],
                                         f12[0:1, j, :lw],
                                         start=True, stop=True)
                        nc.vector.tensor_copy(fb[:, j, :lw], psb[:, :lw])
                    for mc, (vs0, rows) in enumerate(VS_CHUNKS):
                        nc.vector.tensor_tensor(
                            exs[:rows, mc, :lw], exs[:rows, mc, :lw],
                            fb[:rows, 0, :lw], mybir.AluOpType.mult)
                        nc.vector.tensor_tensor(
                            exc[:rows, mc, :lw], exc[:rows, mc, :lw],
                            fb[:rows, 1, :lw], mybir.AluOpType.mult)

                    # memT = sv^T @ exp_s + vc^T @ exp_c (one PSUM group)
                    memt = work.tile([P, CO, 340], MD, tag="bigCL_a")
                    for o in range(CO):
                        ps = psp.tile([P, 340], F32, tag="ps")
                        for mc, (vs0, rows) in enumerate(VS_CHUNKS):
                            nc.tensor.matmul(
                                ps[:, :lw], sv[:rows, mc, bass.ts(o, P)],
                                exs[:rows, mc, :lw],
                                start=(mc == 0), stop=False)
                        for mc, (vs0, rows) in enumerate(VS_CHUNKS):
                            nc.tensor.matmul(
                                ps[:, :lw], vct[:rows, mc, bass.ts(o, P)],
                                exc[:rows, mc, :lw],
                                start=False, stop=(mc == 2))
                        nc.vector.tensor_copy(memt[:, o, :lw], ps[:, :lw])

                    # final projections
                    mk1 = evq.tile([P, 340], MD, tag="mk1")
                    ps1 = psp.tile([P, 340], F32, tag="ps")
                    for k in range(CO):
                        nc.tensor.matmul(ps1[:, :lw], kw1v1_sb[:, k, :],
                                         memt[:, k, :lw],
                                         start=(k == 0), stop=(k == CO - 1))
                    nc.scalar.activation(mk1[:, :lw], ps1[:, :lw],
                                         mybir.ActivationFunctionType.Identity,
                                         bias=kb1v1_sb[:])
                    for o in range(CO):
                        psk = psp.tile([P, 340], F32, tag="ps")
                        nc.tensor.matmul(psk[:, :lw],
                                         kw2v2_sb[0:64, bass.ts(o, P)],
                                         mk1[0:64, :lw],
                                         start=True, stop=True)
                        ok = evp.tile([P, 340], F32, tag="outev")
                        nc.scalar.activation(
                            ok[:, :lw], psk[:, :lw],
                            mybir.ActivationFunctionType.Identity,
                            bias=kb2s_sb[:, o:o + 1])
                        nc.sync.dma_start(
                            r3(mkt[b])[:, o, n0:n0 + lw], ok[:, :lw])
                        psv = psp.tile([P, 340], F32, tag="ps")
                        nc.tensor.matmul(psv[:, :lw],
                                         kw2v2_sb[64:128, bass.ts(o, P)],
                                         mk1[64:128, :lw],
                                         start=True, stop=True)
                        ov = evp.tile([P, 340], F32, tag="outev")
                        nc.scalar.activation(
                            ov[:, :lw], psv[:, :lw],
                            mybir.ActivationFunctionType.Identity,
                            bias=vb2s_sb[:, o:o + 1])
                        nc.sync.dma_start(
                            r3(mvt[b])[:, o, n0:n0 + lw], ov[:, :lw])
    nc.compile()
    return nc


def kernel(x, category_ids, shared_memory, cat_A, cat_B, cat_emb, scale_emb,
           Wq, Wk, Wv, aW1, ab1, aW2, ab2,
           kW1, kb1, kW2, kb2, vW1, vb1, vW2, vb2,
           gk_logit, gv_logit, log_temp):
    f32 = np.float32
    x = np.asarray(x, f32)
    category_ids = np.asarray(category_ids)
    shared_memory = np.asarray(shared_memory, f32)
    cat_A = np.asarray(cat_A, f32)
    cat_B = np.asarray(cat_B, f32)
    cat_emb = np.asarray(cat_emb, f32)
    scale_emb = np.asarray(scale_emb, f32)
    Wq, Wk, Wv = (np.asarray(a, f32) for a in (Wq, Wk, Wv))
    aW1, ab1 = np.asarray(aW1, f32), np.asarray(ab1, f32)
    aW2, ab2 = np.asarray(aW2, f32), np.asarray(ab2, f32)
    kW1, kb1, kW2, kb2 = (np.asarray(a, f32) for a in (kW1, kb1, kW2, kb2))
    vW1, vb1, vW2, vb2 = (np.asarray(a, f32) for a in (vW1, vb1, vW2, vb2))

    temp = float(np.clip(np.exp(np.asarray(log_temp, f32)), 0.05, 1.0))
    sc_scale = float((1.0 / math.sqrt(C)) / temp)
    sig_gk = float(1.0 / (1.0 + np.exp(-np.asarray(gk_logit, f32))))
    sig_gv = float(1.0 / (1.0 + np.exp(-np.asarray(gv_logit, f32))))

    valid = (category_ids >= 0)
    cid = np.clip(category_ids, 0, None).astype(np.int64)
    a_all = cat_A[cid]  # [B, S, R]
    ce_alpha = cat_emb[cid] @ aW1[C:2 * C]  # [B, AH]
    se_alpha = scale_emb @ aW1[2 * C:3 * C]  # [S, AH]

    bf16 = ml_dtypes.bfloat16
    smt = np.ascontiguousarray(shared_memory.reshape(VS, C).T)
    catb = np.ascontiguousarray(cat_B.reshape(S * R, SLOTS * C)).astype(bf16)
    kw1v1 = np.ascontiguousarray(np.concatenate([kW1, vW1], axis=1))
    kb1v1 = np.concatenate([kb1, vb1]).reshape(128, 1).astype(f32)
    kw2v2 = np.ascontiguousarray(
        np.concatenate([sig_gk * kW2, sig_gv * vW2], axis=0))
    kb2s = np.ascontiguousarray(
        (sig_gk * kb2).reshape(CO, P).T).astype(f32)  # [P, CO]
    vb2s = np.ascontiguousarray((sig_gv * vb2).reshape(CO, P).T).astype(f32)

    # per-core tensors
    in_maps = []
    ab2_all = []
    for core in range(NCORES):
        bs = slice(core * BL, (core + 1) * BL)
        xt = np.ascontiguousarray(x[bs].transpose(0, 2, 1))
        abd = np.zeros((S * R, BL * S), dtype=f32)
        for b in range(BL):
            for v in range(S):
                abd[v * R:(v + 1) * R, b * S + v] = a_all[bs][b, v]
        biasall = (ce_alpha[bs][:, None, :] + se_alpha[None, :, :]
                   + ab1[None, None, :])  # [BL, S, AH]
        biasall = np.ascontiguousarray(biasall.transpose(2, 0, 1)).astype(f32)
        ab2_b = tuple(float(ab2[0]) if valid[bs][b] else -30.0
                      for b in range(BL))
        ab2_all.append(ab2_b)
        md = bf16 if MD == BF16 else f32

        in_maps.append({
            "xt": xt.astype(md), "smt": smt.astype(md),
            "wq": Wq.astype(md), "wk": Wk.astype(md), "wv": Wv.astype(md),
            "wkb": Wk.astype(bf16), "wvb": Wv.astype(bf16),
            "catb": catb, "abd": abd.astype(bf16),
            "aw1x": np.ascontiguousarray(aW1[:C]).astype(md),
            "aw2": aW2.astype(md),
            "biasall": biasall, "kw1v1": kw1v1.astype(md), "kb1v1": kb1v1,
            "kw2v2": kw2v2.astype(md), "kb2s": kb2s, "vb2s": vb2s,
            "ab2in": np.array([ab2_b], dtype=f32),
            "onescol": np.ones((128, 1), md),
            "onesrow": np.ones((1, 128), md),
        })

    global _last_in_maps
    _last_in_maps = in_maps
    key = sc_scale
    if key not in _nc_cache:
        _nc_cache.clear()
        _nc_cache[key] = _build(sc_scale)
    ncs = _nc_cache[key]
    res = run_bass_kernel_spmd(ncs, in_maps, core_ids=list(range(NCORES)))

    mem_k = np.empty((B, L, C), f32)
    mem_v = np.empty((B, L, C), f32)
    for core in range(NCORES):
        r = res.results[core]
        bs = slice(core * BL, (core + 1) * BL)
        mem_k[bs] = r["mkt"].transpose(0, 2, 1)
        mem_v[bs] = r["mvt"].transpose(0, 2, 1)
    zero = np.float32(0.0)
    return mem_k, mem_v, zero, zero
